# revision 9
# baseline (speedup 1.0000x reference)
"""Trainium2 kernel for nn_K_graph (gnn_message_passing).

Strategy (per sharding_hint): the heavy [C,B,B] per-graph adjacency work
(S = pm pm^T, masked exp softmax, degree norm, 2 GCN propagation layers with
masked layernorm) runs on 8 NeuronCores, 4 graphs per core (C=32 sharded).
The tiny front (feature embedding, importance MLP, top-K) and tail
(gather + prediction MLP) are pre/post processing around the device kernel.
"""
import sys, os
sys.path.insert(0, "/opt/trn_rl_repo")
import numpy as np

B, NN, NC, H, V, K = 1024, 16, 16, 64, 100, 8
C = NN + NC
NEG = -1e9
NCORE = 8
GPC = C // NCORE  # graphs per core = 4
IB = B // 128     # 8 partition blocks

F32 = np.float32


# ---------------- host front (numpy mirror of reference front) -------------
def _ln_all(x, eps=1e-5):
    mu = x.mean()
    var = ((x - mu) ** 2).mean()
    return (x - mu) / np.sqrt(var + eps)


def _ln_last(x, g, b, eps=1e-5):
    mu = x.mean(-1, keepdims=True)
    var = ((x - mu) ** 2).mean(-1, keepdims=True)
    return (x - mu) / np.sqrt(var + eps) * g + b


def _front(num_data, cat_data, num_w, num_b, cat_emb, fi_w1, fi_b1, fi_g,
           fi_be, fi_w2, fi_b2, gcn1_w):
    fe_num = num_data[..., None] * num_w[None] + num_b[None]
    fe_num = _ln_all(np.maximum(fe_num.reshape(B, NN * H), 0.0))
    fe_cat = cat_emb[np.arange(NC)[None, :], cat_data]
    fe_cat = _ln_all(fe_cat.reshape(B, NC * H))
    feat = np.concatenate([fe_num, fe_cat], axis=1).astype(F32)
    fe3 = feat.reshape(B, C, H)
    h = np.maximum(fe3 @ fi_w1 + fi_b1, 0.0)
    h = _ln_last(h, fi_g, fi_be)
    imp = _ln_all((h @ fi_w2 + fi_b2)[..., 0]).astype(F32)   # [B,C]
    fe3 = (fe3 * imp[..., None]).astype(F32)
    feat = fe3.reshape(B, C * H)
    # top-K per row
    idx = np.argsort(-imp, axis=1, kind="stable")[:, :K]      # [B,K]
    mask = np.zeros((B, C), F32)
    np.put_along_axis(mask, idx, 1.0, axis=1)
    z = np.where(mask > 0, imp, NEG)
    z = z - z.max(1, keepdims=True)
    e = np.exp(z)
    p = (e / e.sum(1, keepdims=True)) * mask                  # [B,C]
    mT = mask.T.copy()                                        # [C,B]
    pm = p[None, :, :] * mT[:, :, None] * (1.0 - np.eye(C, dtype=F32))[:, None, :]
    Y1 = (feat @ gcn1_w).astype(F32)                          # [B,H]
    return fe3, idx, mT, pm.astype(F32), Y1


# ---------------- numpy middle (validation / fallback) ---------------------
def _middle_np(pm, mT, Y1, gcn1_b, gcn2_w, gcn2_b):
    xs = np.zeros((C, B, H), F32)
    for c in range(C):
        M = pm[c]                               # [B,C]
        S = (M @ M.T) * (1.0 - np.eye(B, dtype=F32))
        Ffull = np.exp(S)
        E = (S > 0).astype(F32) * Ffull
        rs = E.sum(1)
        Z = rs.sum()
        Zg = Z + (1.0 if Z <= 0 else 0.0)
        invZ = 1.0 / Zg
        m = mT[c]
        deg = rs * invZ + m
        dinv = 1.0 / np.sqrt(deg + 1.0 - m) * m
        x = Y1
        for (W, bvec) in ((None, gcn1_b), (gcn2_w, gcn2_b)):
            Yin = x if W is None else x @ W
            Ydn = dinv[:, None] * Yin
            u = E @ Ydn
            xl = dinv[:, None] * (u * invZ + m[:, None] * Ydn) + bvec
            r = np.maximum(xl, 0.0)
            rm = r * m[:, None]
            cnt = max(m.sum() * H, 1.0)
            mu = rm.sum() / cnt
            var = (rm * rm).sum() / cnt - mu * mu
            x = (r - mu) / np.sqrt(var + 1e-5)
        xs[c] = x
    return xs


# ---------------- device kernel -------------------------------------------
def _build_device():
    from concourse import bacc, tile
    import concourse.bass as bass
    import concourse.mybir as mybir
    dt = mybir.dt.float32
    ALU = mybir.AluOpType
    ACT = mybir.ActivationFunctionType
    AX = mybir.AxisListType

    nc = bacc.Bacc(None, target_bir_lowering=False, debug=False)
    pmT_d = nc.declare_dram_parameter("pmT", [GPC, C, B], dt, isOutput=False)
    mTp_d = nc.declare_dram_parameter("mTp", [GPC, 128, IB], dt, isOutput=False)
    y1_d = nc.declare_dram_parameter("y1p", [128, IB, H], dt, isOutput=False)
    w2_d = nc.declare_dram_parameter("w2", [H, H], dt, isOutput=False)
    b1_d = nc.declare_dram_parameter("b1bc", [128, H], dt, isOutput=False)
    b2_d = nc.declare_dram_parameter("b2bc", [128, H], dt, isOutput=False)
    eyec_d = nc.declare_dram_parameter("eyec", [128, 128], dt, isOutput=False)
    eye_d = nc.declare_dram_parameter("eye", [128, 128], dt, isOutput=False)
    out_d = nc.declare_dram_parameter("xout", [GPC, B, H], dt, isOutput=True)

    with tile.TileContext(nc) as tc:
        with (
            tc.tile_pool(name="const", bufs=1) as cpool,
            tc.tile_pool(name="big", bufs=1) as bigp,
            tc.tile_pool(name="work", bufs=2) as wp,
            tc.tile_pool(name="scal", bufs=2) as sp,
            tc.tile_pool(name="ps", bufs=2, space=bass.MemorySpace.PSUM) as psp,
            tc.tile_pool(name="psu", bufs=1, space=bass.MemorySpace.PSUM) as psu,
        ):
            y1_sb = cpool.tile([128, IB, H], dt)
            w2_sb = cpool.tile([H, H], dt)
            b1_sb = cpool.tile([128, H], dt)
            b2_sb = cpool.tile([128, H], dt)
            eyec_sb = cpool.tile([128, 128], dt)
            eye_sb = cpool.tile([128, 128], dt)
            ones_c = cpool.tile([128, 1], dt)
            ones_r = cpool.tile([1, 128], dt)
            nc.sync.dma_start(y1_sb[:], y1_d[:])
            nc.sync.dma_start(w2_sb[:], w2_d[:])
            nc.sync.dma_start(b1_sb[:], b1_d[:])
            nc.sync.dma_start(b2_sb[:], b2_d[:])
            nc.sync.dma_start(eyec_sb[:], eyec_d[:])
            nc.sync.dma_start(eye_sb[:], eye_d[:])
            nc.vector.memset(ones_c[:], 1.0)
            nc.vector.memset(ones_r[:], 1.0)

            def bscalar(src_11):
                """broadcast [1,1] sbuf scalar -> [128,1] sbuf"""
                ps = psu.tile([128, 1], dt, tag="mm_ps", bufs=2)
                nc.tensor.matmul(ps[:], ones_r[:], src_11, start=True, stop=True)
                sb = sp.tile([128, 1], dt, tag="bsc")
                nc.vector.tensor_copy(sb[:], ps[:])
                return sb

            def psum_total(col_128_1, tag):
                """sum a [128,1] over partitions -> [1,1] sbuf"""
                ps = psu.tile([1, 1], dt, tag="mm_ps", bufs=2)
                nc.tensor.matmul(ps[:], col_128_1, ones_c[:], start=True, stop=True)
                sb = sp.tile([1, 1], dt, tag=tag)
                nc.vector.tensor_copy(sb[:], ps[:])
                return sb

            for g in range(GPC):
                pmT_sb = wp.tile([C, B], dt, tag="pmT")
                mT_sb = wp.tile([128, IB], dt, tag="mT")
                nc.sync.dma_start(pmT_sb[:], pmT_d[g])
                nc.sync.dma_start(mT_sb[:], mTp_d[g])

                E_sb = bigp.tile([128, IB, B], dt, tag="E")
                rs = wp.tile([128, IB], dt, tag="rs")
                for ib in range(IB):
                    s_ps = psp.tile([128, B], dt, tag="spsum")
                    lhs = pmT_sb[:, ib * 128:(ib + 1) * 128]
                    nc.tensor.matmul(s_ps[:, 0:512], lhs, pmT_sb[:, 0:512],
                                     start=True, stop=True)
                    nc.tensor.matmul(s_ps[:, 512:1024], lhs, pmT_sb[:, 512:1024],
                                     start=True, stop=True)
                    # zero the diagonal slab
                    nc.vector.tensor_mul(s_ps[:, ib * 128:(ib + 1) * 128],
                                         s_ps[:, ib * 128:(ib + 1) * 128],
                                         eyec_sb[:])
                    f_sb = wp.tile([128, B], dt, tag="F")
                    nc.scalar.activation(f_sb[:], s_ps[:], ACT.Exp)
                    # E = (S>0)*exp(S), rowsum into rs
                    nc.vector.scalar_tensor_tensor(
                        E_sb[:, ib, :], s_ps[:], 0.0, f_sb[:],
                        ALU.is_gt, ALU.mult,
                        accum_out=rs[:, ib:ib + 1])

                # ---- degree / normalization scalars ----
                rsr = sp.tile([128, 1], dt, tag="rsr")
                nc.vector.tensor_reduce(rsr[:], rs[:], AX.X, ALU.add)
                z_sb = psum_total(rsr[:], "z")
                zi = sp.tile([1, 1], dt, tag="zi")
                nc.vector.tensor_scalar(zi[:], z_sb[:], 0.0, None, ALU.is_le)
                zg = sp.tile([1, 1], dt, tag="zg")
                nc.vector.tensor_add(zg[:], z_sb[:], zi[:])
                zbc = bscalar(zg[:])
                invz = sp.tile([128, 1], dt, tag="invz")
                nc.vector.reciprocal(invz[:], zbc[:])

                deg = wp.tile([128, IB], dt, tag="deg")
                nc.vector.scalar_tensor_tensor(deg[:], rs[:], invz[:, 0:1],
                                               mT_sb[:], ALU.mult, ALU.add)
                mtm1 = wp.tile([128, IB], dt, tag="mtm1")
                nc.vector.tensor_scalar_sub(mtm1[:], mT_sb[:], 1.0)
                degg = wp.tile([128, IB], dt, tag="degg")
                nc.vector.tensor_sub(degg[:], deg[:], mtm1[:])
                dsq = wp.tile([128, IB], dt, tag="dsq")
                nc.scalar.activation(dsq[:], degg[:], ACT.Sqrt)
                draw = wp.tile([128, IB], dt, tag="draw")
                nc.vector.reciprocal(draw[:], dsq[:])
                dinv = wp.tile([128, IB], dt, tag="dinv")
                nc.vector.tensor_mul(dinv[:], draw[:], mT_sb[:])

                def gcn_layer(ydn, b_sb, out_r):
                    """ydn: [128,IB,H] = dinv*Yin already; returns relu'd+ln'd"""
                    s1 = wp.tile([128, IB], dt, tag="s1")
                    s2 = wp.tile([128, IB], dt, tag="s2")
                    r_sb = out_r
                    for ib in range(IB):
                        mydn = wp.tile([128, H], dt, tag="mydn")
                        nc.vector.tensor_scalar_mul(mydn[:], ydn[:, ib, :],
                                                    mT_sb[:, ib:ib + 1])
                        u_ps = psu.tile([128, H], dt, tag="ups", bufs=2)
                        for jb in range(IB):
                            nc.tensor.matmul(
                                u_ps[:],
                                E_sb[:, jb, ib * 128:(ib + 1) * 128],
                                ydn[:, jb, :],
                                start=(jb == 0), stop=(jb == IB - 1))
                        t_sb = wp.tile([128, H], dt, tag="t")
                        nc.vector.scalar_tensor_tensor(
                            t_sb[:], u_ps[:], invz[:, 0:1], mydn[:],
                            ALU.mult, ALU.add)
                        v_sb = wp.tile([128, H], dt, tag="v")
                        nc.vector.scalar_tensor_tensor(
                            v_sb[:], t_sb[:], dinv[:, ib:ib + 1], b_sb[:],
                            ALU.mult, ALU.add)
                        nc.scalar.activation(r_sb[:, ib, :], v_sb[:], ACT.Relu)
                        rm = wp.tile([128, H], dt, tag="rm")
                        nc.vector.tensor_scalar(rm[:], r_sb[:, ib, :],
                                                mT_sb[:, ib:ib + 1], 0.0,
                                                ALU.mult, ALU.add,
                                                accum_out=s1[:, ib:ib + 1])
                        sqs = wp.tile([128, H], dt, tag="sqs")
                        nc.scalar.activation(sqs[:], rm[:], ACT.Square,
                                             accum_out=s2[:, ib:ib + 1])
                    # masked layernorm stats
                    s1r = sp.tile([128, 1], dt, tag="s1r")
                    s2r = sp.tile([128, 1], dt, tag="s2r")
                    mtr = sp.tile([128, 1], dt, tag="mtr")
                    nc.vector.tensor_reduce(s1r[:], s1[:], AX.X, ALU.add)
                    nc.vector.tensor_reduce(s2r[:], s2[:], AX.X, ALU.add)
                    nc.vector.tensor_reduce(mtr[:], mT_sb[:], AX.X, ALU.add)
                    s1t = psum_total(s1r[:], "s1t")
                    s2t = psum_total(s2r[:], "s2t")
                    mtt = psum_total(mtr[:], "mtt")
                    cnt = sp.tile([1, 1], dt, tag="cnt")
                    nc.vector.tensor_scalar_mul(cnt[:], mtt[:], float(H))
                    ci = sp.tile([1, 1], dt, tag="ci")
                    nc.vector.tensor_scalar(ci[:], cnt[:], 0.0, None, ALU.is_le)
                    cg = sp.tile([1, 1], dt, tag="cg")
                    nc.vector.tensor_add(cg[:], cnt[:], ci[:])
                    icnt1 = sp.tile([1, 1], dt, tag="icnt1")
                    nc.vector.reciprocal(icnt1[:], cg[:])
                    icb = bscalar(icnt1[:])
                    s1b = bscalar(s1t[:])
                    s2b = bscalar(s2t[:])
                    mu = sp.tile([128, 1], dt, tag="mu")
                    nc.vector.tensor_mul(mu[:], s1b[:], icb[:])
                    musq = sp.tile([128, 1], dt, tag="musq")
                    nc.vector.tensor_mul(musq[:], mu[:], mu[:])
                    var = sp.tile([128, 1], dt, tag="var")
                    nc.vector.scalar_tensor_tensor(var[:], s2b[:], icb[:, 0:1],
                                                   musq[:], ALU.mult,
                                                   ALU.subtract)
                    vare = sp.tile([128, 1], dt, tag="vare")
                    nc.vector.tensor_scalar_add(vare[:], var[:], 1e-5)
                    sig = sp.tile([128, 1], dt, tag="sig")
                    nc.scalar.activation(sig[:], vare[:], ACT.Sqrt)
                    rsig = sp.tile([128, 1], dt, tag="rsig")
                    nc.vector.reciprocal(rsig[:], sig[:])
                    nmu = sp.tile([128, 1], dt, tag="nmu")
                    nc.vector.tensor_scalar_mul(nmu[:], mu[:], -1.0)
                    for ib in range(IB):
                        nc.vector.tensor_scalar(r_sb[:, ib, :], r_sb[:, ib, :],
                                                nmu[:, 0:1], rsig[:, 0:1],
                                                ALU.add, ALU.mult)

                # ---- layer 1 ----
                ydn1 = wp.tile([128, IB, H], dt, tag="ydn1")
                for ib in range(IB):
                    nc.vector.tensor_scalar_mul(ydn1[:, ib, :], y1_sb[:, ib, :],
                                                dinv[:, ib:ib + 1])
                x1n = wp.tile([128, IB, H], dt, tag="x1n")
                gcn_layer(ydn1, b1_sb, x1n)

                # ---- transpose x1n, Y2 = x1n @ W2, ydn2 fused ----
                x1nT = wp.tile([H, B], dt, tag="x1nT")
                for ib in range(IB):
                    tp = psu.tile([H, 128], dt, tag="mm_ps", bufs=2)
                    nc.tensor.transpose(tp[:], x1n[:, ib, :], eye_sb[:])
                    nc.vector.tensor_copy(x1nT[:, ib * 128:(ib + 1) * 128], tp[:])
                ydn2 = wp.tile([128, IB, H], dt, tag="ydn2")
                for ib in range(IB):
                    y2ps = psu.tile([128, H], dt, tag="mm_ps", bufs=2)
                    nc.tensor.matmul(y2ps[:], x1nT[:, ib * 128:(ib + 1) * 128],
                                     w2_sb[:], start=True, stop=True)
                    nc.vector.tensor_scalar_mul(ydn2[:, ib, :], y2ps[:],
                                                dinv[:, ib:ib + 1])
                x2n = wp.tile([128, IB, H], dt, tag="x2n")
                gcn_layer(ydn2, b2_sb, x2n)
                nc.sync.dma_start(
                    out_d[g].rearrange("(b p) h -> p b h", p=128), x2n[:])
    nc.finalize()
    return nc


_NC_CACHE = None


def _middle_device(pm, mT, Y1, gcn1_b, gcn2_w, gcn2_b):
    global _NC_CACHE
    from concourse.bass_utils import run_bass_kernel_spmd
    if _NC_CACHE is None:
        _NC_CACHE = _build_device()
    nc = _NC_CACHE
    eyec = (1.0 - np.eye(128)).astype(F32)
    eye = np.eye(128, dtype=F32)
    y1p = np.ascontiguousarray(Y1.reshape(IB, 128, H).transpose(1, 0, 2))
    b1bc = np.tile(gcn1_b[None, :], (128, 1)).astype(F32)
    b2bc = np.tile(gcn2_b[None, :], (128, 1)).astype(F32)
    in_maps = []
    for r in range(NCORE):
        cs = slice(r * GPC, (r + 1) * GPC)
        pmT = np.ascontiguousarray(pm[cs].transpose(0, 2, 1))      # [4,C,B]
        mTp = np.ascontiguousarray(
            mT[cs].reshape(GPC, IB, 128).transpose(0, 2, 1))       # [4,128,IB]
        in_maps.append(dict(pmT=pmT, mTp=mTp, y1p=y1p, w2=gcn2_w.astype(F32),
                            b1bc=b1bc, b2bc=b2bc, eyec=eyec, eye=eye))
    res = run_bass_kernel_spmd(nc, in_maps, list(range(NCORE)))
    xs = np.concatenate([np.asarray(res.results[r]["xout"])
                         for r in range(NCORE)], axis=0)           # [C,B,H]
    return xs.astype(F32)


# ---------------- public entry --------------------------------------------
def kernel(num_data, cat_data, num_w, num_b, cat_emb, fi_w1, fi_b1, fi_g,
           fi_be, fi_w2, fi_b2, gcn1_w, gcn1_b, gcn2_w, gcn2_b, pw1, pb1,
           pg, pbe, pw2, pb2):
    args = [np.asarray(a) for a in (num_data, cat_data, num_w, num_b, cat_emb,
                                    fi_w1, fi_b1, fi_g, fi_be, fi_w2, fi_b2,
                                    gcn1_w)]
    fe3, idx, mT, pm, Y1 = _front(*args)
    if os.environ.get("KG_NUMPY"):
        xs = _middle_np(pm, mT, Y1, np.asarray(gcn1_b), np.asarray(gcn2_w),
                        np.asarray(gcn2_b))
    else:
        try:
            xs = _middle_device(pm, mT, Y1, np.asarray(gcn1_b),
                                np.asarray(gcn2_w), np.asarray(gcn2_b))
        except Exception as ex:  # safety net: never return garbage
            print(f"[kernel] device path failed ({ex!r}); numpy fallback",
                  file=sys.stderr)
            xs = _middle_np(pm, mT, Y1, np.asarray(gcn1_b), np.asarray(gcn2_w),
                            np.asarray(gcn2_b))
    cols = np.sort(idx, axis=1)
    gathered = xs[cols, np.arange(B)[:, None]]                     # [B,K,H]
    full = np.concatenate([gathered, fe3], axis=1).reshape(B, (K + C) * H)
    h = _ln_last(np.maximum(full @ np.asarray(pw1) + np.asarray(pb1), 0.0),
                 np.asarray(pg), np.asarray(pbe))
    out = h @ np.asarray(pw2) + np.asarray(pb2)
    return out.astype(F32)



# revision 11
# speedup vs baseline: 1731.3179x; 1731.3179x over previous
"""Trainium2 kernel for nn_K_graph (gnn_message_passing).

Strategy (per sharding_hint): the heavy [C,B,B] per-graph adjacency work
(S = pm pm^T, masked exp softmax, degree norm, 2 GCN propagation layers with
masked layernorm) runs on 8 NeuronCores, 4 graphs per core (C=32 sharded).
The tiny front (feature embedding, importance MLP, top-K) and tail
(gather + prediction MLP) are pre/post processing around the device kernel.
"""
import sys, os
sys.path.insert(0, "/opt/trn_rl_repo")
import numpy as np

B, NN, NC, H, V, K = 1024, 16, 16, 64, 100, 8
C = NN + NC
NEG = -1e9
NCORE = 8
GPC = C // NCORE  # graphs per core = 4
IB = B // 128     # 8 partition blocks

F32 = np.float32


# ---------------- host front (numpy mirror of reference front) -------------
def _ln_all(x, eps=1e-5):
    mu = x.mean()
    var = ((x - mu) ** 2).mean()
    return (x - mu) / np.sqrt(var + eps)


def _ln_last(x, g, b, eps=1e-5):
    mu = x.mean(-1, keepdims=True)
    var = ((x - mu) ** 2).mean(-1, keepdims=True)
    return (x - mu) / np.sqrt(var + eps) * g + b


def _front(num_data, cat_data, num_w, num_b, cat_emb, fi_w1, fi_b1, fi_g,
           fi_be, fi_w2, fi_b2, gcn1_w):
    fe_num = num_data[..., None] * num_w[None] + num_b[None]
    fe_num = _ln_all(np.maximum(fe_num.reshape(B, NN * H), 0.0))
    fe_cat = cat_emb[np.arange(NC)[None, :], cat_data]
    fe_cat = _ln_all(fe_cat.reshape(B, NC * H))
    feat = np.concatenate([fe_num, fe_cat], axis=1).astype(F32)
    fe3 = feat.reshape(B, C, H)
    h = np.maximum(fe3 @ fi_w1 + fi_b1, 0.0)
    h = _ln_last(h, fi_g, fi_be)
    imp = _ln_all((h @ fi_w2 + fi_b2)[..., 0]).astype(F32)   # [B,C]
    fe3 = (fe3 * imp[..., None]).astype(F32)
    feat = fe3.reshape(B, C * H)
    # top-K per row
    idx = np.argsort(-imp, axis=1, kind="stable")[:, :K]      # [B,K]
    mask = np.zeros((B, C), F32)
    np.put_along_axis(mask, idx, 1.0, axis=1)
    z = np.where(mask > 0, imp, NEG)
    z = z - z.max(1, keepdims=True)
    e = np.exp(z)
    p = (e / e.sum(1, keepdims=True)) * mask                  # [B,C]
    mT = mask.T.copy()                                        # [C,B]
    pm = p[None, :, :] * mT[:, :, None] * (1.0 - np.eye(C, dtype=F32))[:, None, :]
    Y1 = (feat @ gcn1_w).astype(F32)                          # [B,H]
    return fe3, idx, mT, pm.astype(F32), Y1


# ---------------- numpy middle (validation / fallback) ---------------------
def _middle_np(pm, mT, Y1, gcn1_b, gcn2_w, gcn2_b):
    xs = np.zeros((C, B, H), F32)
    for c in range(C):
        M = pm[c]                               # [B,C]
        S = (M @ M.T) * (1.0 - np.eye(B, dtype=F32))
        Ffull = np.exp(S)
        E = (S > 0).astype(F32) * Ffull
        rs = E.sum(1)
        Z = rs.sum()
        Zg = Z + (1.0 if Z <= 0 else 0.0)
        invZ = 1.0 / Zg
        m = mT[c]
        deg = rs * invZ + m
        dinv = 1.0 / np.sqrt(deg + 1.0 - m) * m
        x = Y1
        for (W, bvec) in ((None, gcn1_b), (gcn2_w, gcn2_b)):
            Yin = x if W is None else x @ W
            Ydn = dinv[:, None] * Yin
            u = E @ Ydn
            xl = dinv[:, None] * (u * invZ + m[:, None] * Ydn) + bvec
            r = np.maximum(xl, 0.0)
            rm = r * m[:, None]
            cnt = max(m.sum() * H, 1.0)
            mu = rm.sum() / cnt
            var = (rm * rm).sum() / cnt - mu * mu
            x = (r - mu) / np.sqrt(var + 1e-5)
        xs[c] = x
    return xs


# ---------------- device kernel -------------------------------------------
def _build_device():
    from concourse import bacc, tile
    import concourse.bass as bass
    import concourse.mybir as mybir
    dt = mybir.dt.float32
    ALU = mybir.AluOpType
    ACT = mybir.ActivationFunctionType
    AX = mybir.AxisListType

    nc = bacc.Bacc(None, target_bir_lowering=False, debug=False)
    pmT_d = nc.declare_dram_parameter("pmT", [GPC, C, B], dt, isOutput=False)
    mTp_d = nc.declare_dram_parameter("mTp", [GPC, 128, IB], dt, isOutput=False)
    y1_d = nc.declare_dram_parameter("y1p", [128, IB, H], dt, isOutput=False)
    w2_d = nc.declare_dram_parameter("w2", [H, H], dt, isOutput=False)
    b1_d = nc.declare_dram_parameter("b1bc", [128, H], dt, isOutput=False)
    b2_d = nc.declare_dram_parameter("b2bc", [128, H], dt, isOutput=False)
    eyec_d = nc.declare_dram_parameter("eyec", [128, 128], dt, isOutput=False)
    eye_d = nc.declare_dram_parameter("eye", [128, 128], dt, isOutput=False)
    out_d = nc.declare_dram_parameter("xout", [GPC, B, H], dt, isOutput=True)

    with tile.TileContext(nc) as tc:
        with (
            tc.tile_pool(name="const", bufs=1) as cpool,
            tc.tile_pool(name="big", bufs=1) as bigp,
            tc.tile_pool(name="work", bufs=2) as wp,
            tc.tile_pool(name="scal", bufs=2) as sp,
            tc.tile_pool(name="ps", bufs=2, space=bass.MemorySpace.PSUM) as psp,
            tc.tile_pool(name="psu", bufs=1, space=bass.MemorySpace.PSUM) as psu,
        ):
            y1_sb = cpool.tile([128, IB, H], dt)
            w2_sb = cpool.tile([H, H], dt)
            b1_sb = cpool.tile([128, H], dt)
            b2_sb = cpool.tile([128, H], dt)
            eyec_sb = cpool.tile([128, 128], dt)
            eye_sb = cpool.tile([128, 128], dt)
            ones_c = cpool.tile([128, 1], dt)
            ones_r = cpool.tile([1, 128], dt)
            nc.sync.dma_start(y1_sb[:], y1_d[:])
            nc.sync.dma_start(w2_sb[:], w2_d[:])
            nc.sync.dma_start(b1_sb[:], b1_d[:])
            nc.sync.dma_start(b2_sb[:], b2_d[:])
            nc.sync.dma_start(eyec_sb[:], eyec_d[:])
            nc.sync.dma_start(eye_sb[:], eye_d[:])
            nc.vector.memset(ones_c[:], 1.0)
            nc.vector.memset(ones_r[:], 1.0)

            def bscalar(src_11):
                """broadcast [1,1] sbuf scalar -> [128,1] sbuf"""
                ps = psu.tile([128, 1], dt, tag="mm_ps", bufs=2)
                nc.tensor.matmul(ps[:], ones_r[:], src_11, start=True, stop=True)
                sb = sp.tile([128, 1], dt, tag="bsc")
                nc.vector.tensor_copy(sb[:], ps[:])
                return sb

            def psum_total(col_128_1, tag):
                """sum a [128,1] over partitions -> [1,1] sbuf"""
                ps = psu.tile([1, 1], dt, tag="mm_ps", bufs=2)
                nc.tensor.matmul(ps[:], col_128_1, ones_c[:], start=True, stop=True)
                sb = sp.tile([1, 1], dt, tag=tag)
                nc.vector.tensor_copy(sb[:], ps[:])
                return sb

            for g in range(GPC):
                pmT_sb = wp.tile([C, B], dt, tag="pmT")
                mT_sb = wp.tile([128, IB], dt, tag="mT")
                nc.sync.dma_start(pmT_sb[:], pmT_d[g])
                nc.sync.dma_start(mT_sb[:], mTp_d[g])

                E_sb = bigp.tile([128, IB, B], dt, tag="E")
                rs = wp.tile([128, IB], dt, tag="rs")
                for ib in range(IB):
                    s_ps = psp.tile([128, B], dt, tag="spsum")
                    lhs = pmT_sb[:, ib * 128:(ib + 1) * 128]
                    nc.tensor.matmul(s_ps[:, 0:512], lhs, pmT_sb[:, 0:512],
                                     start=True, stop=True)
                    nc.tensor.matmul(s_ps[:, 512:1024], lhs, pmT_sb[:, 512:1024],
                                     start=True, stop=True)
                    # zero the diagonal slab
                    nc.vector.tensor_mul(s_ps[:, ib * 128:(ib + 1) * 128],
                                         s_ps[:, ib * 128:(ib + 1) * 128],
                                         eyec_sb[:])
                    f_sb = wp.tile([128, B], dt, tag="F")
                    nc.scalar.activation(f_sb[:], s_ps[:], ACT.Exp)
                    # E = (S>0)*exp(S), rowsum into rs
                    nc.vector.scalar_tensor_tensor(
                        E_sb[:, ib, :], s_ps[:], 0.0, f_sb[:],
                        ALU.is_gt, ALU.mult,
                        accum_out=rs[:, ib:ib + 1])

                # ---- degree / normalization scalars ----
                rsr = sp.tile([128, 1], dt, tag="rsr")
                nc.vector.tensor_reduce(rsr[:], rs[:], AX.X, ALU.add)
                z_sb = psum_total(rsr[:], "z")
                zi = sp.tile([1, 1], dt, tag="zi")
                nc.vector.tensor_scalar(zi[:], z_sb[:], 0.0, None, ALU.is_le)
                zg = sp.tile([1, 1], dt, tag="zg")
                nc.vector.tensor_add(zg[:], z_sb[:], zi[:])
                zbc = bscalar(zg[:])
                invz = sp.tile([128, 1], dt, tag="invz")
                nc.vector.reciprocal(invz[:], zbc[:])

                deg = wp.tile([128, IB], dt, tag="deg")
                nc.vector.scalar_tensor_tensor(deg[:], rs[:], invz[:, 0:1],
                                               mT_sb[:], ALU.mult, ALU.add)
                mtm1 = wp.tile([128, IB], dt, tag="mtm1")
                nc.vector.tensor_scalar_sub(mtm1[:], mT_sb[:], 1.0)
                degg = wp.tile([128, IB], dt, tag="degg")
                nc.vector.tensor_sub(degg[:], deg[:], mtm1[:])
                dsq = wp.tile([128, IB], dt, tag="dsq")
                nc.scalar.activation(dsq[:], degg[:], ACT.Sqrt)
                draw = wp.tile([128, IB], dt, tag="draw")
                nc.vector.reciprocal(draw[:], dsq[:])
                dinv = wp.tile([128, IB], dt, tag="dinv")
                nc.vector.tensor_mul(dinv[:], draw[:], mT_sb[:])

                def gcn_layer(ydn, b_sb, out_r):
                    """ydn: [128,IB,H] = dinv*Yin already; returns relu'd+ln'd"""
                    s1 = wp.tile([128, IB], dt, tag="s1")
                    s2 = wp.tile([128, IB], dt, tag="s2")
                    r_sb = out_r
                    for ib in range(IB):
                        mydn = wp.tile([128, H], dt, tag="mydn")
                        nc.vector.tensor_scalar_mul(mydn[:], ydn[:, ib, :],
                                                    mT_sb[:, ib:ib + 1])
                        u_ps = psu.tile([128, H], dt, tag="ups", bufs=2)
                        for jb in range(IB):
                            nc.tensor.matmul(
                                u_ps[:],
                                E_sb[:, jb, ib * 128:(ib + 1) * 128],
                                ydn[:, jb, :],
                                start=(jb == 0), stop=(jb == IB - 1))
                        t_sb = wp.tile([128, H], dt, tag="t")
                        nc.vector.scalar_tensor_tensor(
                            t_sb[:], u_ps[:], invz[:, 0:1], mydn[:],
                            ALU.mult, ALU.add)
                        v_sb = wp.tile([128, H], dt, tag="v")
                        nc.vector.scalar_tensor_tensor(
                            v_sb[:], t_sb[:], dinv[:, ib:ib + 1], b_sb[:],
                            ALU.mult, ALU.add)
                        nc.scalar.activation(r_sb[:, ib, :], v_sb[:], ACT.Relu)
                        rm = wp.tile([128, H], dt, tag="rm")
                        nc.vector.tensor_scalar(rm[:], r_sb[:, ib, :],
                                                mT_sb[:, ib:ib + 1], 0.0,
                                                ALU.mult, ALU.add,
                                                accum_out=s1[:, ib:ib + 1])
                        sqs = wp.tile([128, H], dt, tag="sqs")
                        nc.scalar.activation(sqs[:], rm[:], ACT.Square,
                                             accum_out=s2[:, ib:ib + 1])
                    # masked layernorm stats
                    s1r = sp.tile([128, 1], dt, tag="s1r")
                    s2r = sp.tile([128, 1], dt, tag="s2r")
                    mtr = sp.tile([128, 1], dt, tag="mtr")
                    nc.vector.tensor_reduce(s1r[:], s1[:], AX.X, ALU.add)
                    nc.vector.tensor_reduce(s2r[:], s2[:], AX.X, ALU.add)
                    nc.vector.tensor_reduce(mtr[:], mT_sb[:], AX.X, ALU.add)
                    s1t = psum_total(s1r[:], "s1t")
                    s2t = psum_total(s2r[:], "s2t")
                    mtt = psum_total(mtr[:], "mtt")
                    cnt = sp.tile([1, 1], dt, tag="cnt")
                    nc.vector.tensor_scalar_mul(cnt[:], mtt[:], float(H))
                    ci = sp.tile([1, 1], dt, tag="ci")
                    nc.vector.tensor_scalar(ci[:], cnt[:], 0.0, None, ALU.is_le)
                    cg = sp.tile([1, 1], dt, tag="cg")
                    nc.vector.tensor_add(cg[:], cnt[:], ci[:])
                    icnt1 = sp.tile([1, 1], dt, tag="icnt1")
                    nc.vector.reciprocal(icnt1[:], cg[:])
                    icb = bscalar(icnt1[:])
                    s1b = bscalar(s1t[:])
                    s2b = bscalar(s2t[:])
                    mu = sp.tile([128, 1], dt, tag="mu")
                    nc.vector.tensor_mul(mu[:], s1b[:], icb[:])
                    musq = sp.tile([128, 1], dt, tag="musq")
                    nc.vector.tensor_mul(musq[:], mu[:], mu[:])
                    var = sp.tile([128, 1], dt, tag="var")
                    nc.vector.scalar_tensor_tensor(var[:], s2b[:], icb[:, 0:1],
                                                   musq[:], ALU.mult,
                                                   ALU.subtract)
                    vare = sp.tile([128, 1], dt, tag="vare")
                    nc.vector.tensor_scalar_add(vare[:], var[:], 1e-5)
                    sig = sp.tile([128, 1], dt, tag="sig")
                    nc.scalar.activation(sig[:], vare[:], ACT.Sqrt)
                    rsig = sp.tile([128, 1], dt, tag="rsig")
                    nc.vector.reciprocal(rsig[:], sig[:])
                    nmu = sp.tile([128, 1], dt, tag="nmu")
                    nc.vector.tensor_scalar_mul(nmu[:], mu[:], -1.0)
                    for ib in range(IB):
                        nc.vector.tensor_scalar(r_sb[:, ib, :], r_sb[:, ib, :],
                                                nmu[:, 0:1], rsig[:, 0:1],
                                                ALU.add, ALU.mult)

                # ---- layer 1 ----
                ydn1 = wp.tile([128, IB, H], dt, tag="ydn1")
                for ib in range(IB):
                    nc.vector.tensor_scalar_mul(ydn1[:, ib, :], y1_sb[:, ib, :],
                                                dinv[:, ib:ib + 1])
                x1n = wp.tile([128, IB, H], dt, tag="x1n")
                gcn_layer(ydn1, b1_sb, x1n)

                # ---- transpose x1n, Y2 = x1n @ W2, ydn2 fused ----
                x1nT = wp.tile([H, B], dt, tag="x1nT")
                for ib in range(IB):
                    tp = psu.tile([H, 128], dt, tag="mm_ps", bufs=2)
                    nc.tensor.transpose(tp[:], x1n[:, ib, :], eye_sb[:])
                    nc.vector.tensor_copy(x1nT[:, ib * 128:(ib + 1) * 128], tp[:])
                ydn2 = wp.tile([128, IB, H], dt, tag="ydn2")
                for ib in range(IB):
                    y2ps = psu.tile([128, H], dt, tag="mm_ps", bufs=2)
                    nc.tensor.matmul(y2ps[:], x1nT[:, ib * 128:(ib + 1) * 128],
                                     w2_sb[:], start=True, stop=True)
                    nc.vector.tensor_scalar_mul(ydn2[:, ib, :], y2ps[:],
                                                dinv[:, ib:ib + 1])
                x2n = wp.tile([128, IB, H], dt, tag="x2n")
                gcn_layer(ydn2, b2_sb, x2n)
                nc.sync.dma_start(
                    out_d[g].rearrange("(b p) h -> p b h", p=128), x2n[:])
    nc.finalize()
    return nc


_NC_CACHE = None
_LAST_EXEC_NS = None
_LAST_TRACE = None


def _middle_device(pm, mT, Y1, gcn1_b, gcn2_w, gcn2_b):
    global _NC_CACHE, _LAST_EXEC_NS, _LAST_TRACE
    from concourse.bass_utils import run_bass_kernel_spmd
    if _NC_CACHE is None:
        _NC_CACHE = _build_device()
    nc = _NC_CACHE
    eyec = (1.0 - np.eye(128)).astype(F32)
    eye = np.eye(128, dtype=F32)
    y1p = np.ascontiguousarray(Y1.reshape(IB, 128, H).transpose(1, 0, 2))
    b1bc = np.tile(gcn1_b[None, :], (128, 1)).astype(F32)
    b2bc = np.tile(gcn2_b[None, :], (128, 1)).astype(F32)
    in_maps = []
    for r in range(NCORE):
        cs = slice(r * GPC, (r + 1) * GPC)
        pmT = np.ascontiguousarray(pm[cs].transpose(0, 2, 1))      # [4,C,B]
        mTp = np.ascontiguousarray(
            mT[cs].reshape(GPC, IB, 128).transpose(0, 2, 1))       # [4,128,IB]
        in_maps.append(dict(pmT=pmT, mTp=mTp, y1p=y1p, w2=gcn2_w.astype(F32),
                            b1bc=b1bc, b2bc=b2bc, eyec=eyec, eye=eye))
    kw = {}
    if os.environ.get("KG_TRACE"):
        kw = dict(trace=True, tmpdir="/tmp/kg_trace")
    res = run_bass_kernel_spmd(nc, in_maps, list(range(NCORE)), **kw)
    if kw:
        _LAST_EXEC_NS = res.exec_time_ns
        _LAST_TRACE = (res.instructions_and_trace[1]
                       if res.instructions_and_trace else None)
        print(f"[kernel] exec_time_ns={_LAST_EXEC_NS} trace={_LAST_TRACE}",
              file=sys.stderr)
    xs = np.concatenate([np.asarray(res.results[r]["xout"])
                         for r in range(NCORE)], axis=0)           # [C,B,H]
    return xs.astype(F32)


# ---------------- public entry --------------------------------------------
def kernel(num_data, cat_data, num_w, num_b, cat_emb, fi_w1, fi_b1, fi_g,
           fi_be, fi_w2, fi_b2, gcn1_w, gcn1_b, gcn2_w, gcn2_b, pw1, pb1,
           pg, pbe, pw2, pb2):
    args = [np.asarray(a) for a in (num_data, cat_data, num_w, num_b, cat_emb,
                                    fi_w1, fi_b1, fi_g, fi_be, fi_w2, fi_b2,
                                    gcn1_w)]
    fe3, idx, mT, pm, Y1 = _front(*args)
    if os.environ.get("KG_NUMPY"):
        xs = _middle_np(pm, mT, Y1, np.asarray(gcn1_b), np.asarray(gcn2_w),
                        np.asarray(gcn2_b))
    else:
        try:
            xs = _middle_device(pm, mT, Y1, np.asarray(gcn1_b),
                                np.asarray(gcn2_w), np.asarray(gcn2_b))
        except Exception as ex:  # safety net: never return garbage
            print(f"[kernel] device path failed ({ex!r}); numpy fallback",
                  file=sys.stderr)
            xs = _middle_np(pm, mT, Y1, np.asarray(gcn1_b), np.asarray(gcn2_w),
                            np.asarray(gcn2_b))
    cols = np.sort(idx, axis=1)
    gathered = xs[cols, np.arange(B)[:, None]]                     # [B,K,H]
    full = np.concatenate([gathered, fe3], axis=1).reshape(B, (K + C) * H)
    h = _ln_last(np.maximum(full @ np.asarray(pw1) + np.asarray(pb1), 0.0),
                 np.asarray(pg), np.asarray(pbe))
    out = h @ np.asarray(pw2) + np.asarray(pb2)
    return out.astype(F32)

